# revision 9
# baseline (speedup 1.0000x reference)
"""DSAttention (de-stationary attention) TRN2 Bass kernel.

Computes, per (b, h):
    scores = (q @ k^T) * tau_b + delta_b          [L, S]
    scores = where(causal_mask, -1e9, scores)
    A = softmax(scale * scores)                    (no max-subtraction needed:
                                                    logits are O(10), exp is safe)
    out = A @ v                                    [L, D]

Strategy: batch*head parallel over 8 cores (4 (b,h) pairs per core).
Per (b,h), everything is computed in "transposed score" space:
    X_T[s, l] = sum_e KT[e, s] * QT[e, l]       (QT pre-scaled by scale*tau on host)
    p = exp(X_T)                                (delta folded in via an aug row:
                                                 KT row 64 = scale*delta, QT row 64 = 1)
    OutT[d, l] = sum_s V'[s, d] * p[s, l]       (V' has a ones column -> row 64 of
                                                 OutT is the softmax denominator)
then the host divides + transposes.

The kernel is ACT(exp)-throughput-bound: the Scalar engine evaluates one exp
per lane per cycle at 1.2 GHz, dtype-independent, and every unmasked score
needs one exp. Everything else is arranged to keep ACT saturated:
  - bf16 data path (matmul runs at full PE rate; halves DMA + SBUF traffic)
  - contraction K=65 (64 e-dims + aug row) -> no zero-pad rows, no memsets,
    input DMA can start immediately and compute starts ~4us in
  - causality exploited at 128-column granularity with NO junk columns:
    the four partial diagonal strips are packed contiguously into one PSUM
    tile (order 512|384|128|256 keeps each matmul inside one 2KB bank)
  - exp is issued over 3-bank PSUM tiles (up to 1536 cols per ACTIVATE,
    52 calls/core instead of 80) to amortize the ~150ns per-call overhead
  - a dummy exp at t=0 pulls the ~2.7us ACT_TABLE_LOAD into the DMA wait
"""

import math

import numpy as np
import ml_dtypes

import bass_rust
import concourse.bass as bass
import concourse.mybir as mybir
import concourse.tile as tile
from concourse.bass_utils import run_bass_kernel_spmd

B, L, S, H, E, D = 2, 2048, 2048, 16, 64, 64
NCORES = 8
BH = B * H                      # 32 (b,h) pairs
BH_PER_CORE = BH // NCORES      # 4
SCALE = 1.0 / math.sqrt(E)

F32 = mybir.dt.float32
BF16 = mybir.dt.bfloat16
NP_BF16 = ml_dtypes.bfloat16

# per-(b,h) input layout:
#   x  [65, 4096]  cols [0, 2048)    rows 0:64 = scale*tau*q^T, row 64 = 1.0
#                  cols [2048, 4096) rows 0:64 = k^T, row 64 = scale*delta
#   v  [128, 1040] V' rearranged: col 65*j + c = V'[128j+p, c]
QK_COLS = 2 * L                 # 4096
VP_COLS = (S // 128) * (D + 1)  # 1040
TILE_W = 1536                   # 3 PSUM banks per score tile


class _SplitDrainTileContext(tile.TileContext):
    """This walrus build rejects instructions carrying more than one sem
    wait; the kernel-tail drain aggregates one wait per active processor.
    Split them across a chain of drains on the same engine."""

    def _drain_and_barrier(self, tick_clock, wait_clock):
        nc = self.nc
        drain_inst = nc.sync.drain()
        wait_clock.add_sem_waits(
            drain_inst.ins, bass_rust.ScopedClock({None: tick_clock.global_clock})
        )
        si = drain_inst.ins.sync_info
        waits = list(si.on_wait) if si is not None and si.on_wait else []
        if len(waits) > 1:
            si.on_wait = waits[:1]
            for w in waits[1:]:
                d2 = nc.sync.drain()
                d2.ins.sync_info = bass_rust.SyncInfo(on_wait=[w], on_update=[])
        nc.all_engine_barrier()
        popped = nc._tile_sem_poison_stack.pop()
        assert popped is self._sem_poison
        nc.clear_and_free_semaphores(list(self.sems.allocated().values()))
        nc.all_engine_barrier()


def _legalize_waits(nc, max_waits=1):
    """This walrus build rejects instructions with more than `max_waits`
    sem waits. Spill extras onto same-engine NoOps inserted just before
    the offending instruction (same-engine program order preserves the
    wait semantics)."""
    for f in nc.m.functions:
        for bb in f.blocks:
            insts = bb.instructions
            for idx in range(len(insts) - 1, -1, -1):
                inst = insts[idx]
                si = getattr(inst, "sync_info", None)
                if si is None or not si.on_wait:
                    continue
                ow = list(si.on_wait)
                sem = [w for w in ow if w.sync_type == "semaphore"]
                other = [w for w in ow if w.sync_type != "semaphore"]
                budget = max(0, max_waits - len(other))
                if len(sem) <= budget:
                    continue
                keep, spill = sem[:budget], sem[budget:]
                si.on_wait = other + keep
                for w in reversed(spill):
                    n = mybir.InstNoOp(name=f"W-{nc.next_id()}", ins=[], outs=[])
                    n.engine = inst.engine
                    n.sync_info = bass_rust.SyncInfo(on_wait=[w], on_update=[])
                    nc.register_instruction(n, overwrite=True)
                    insts.insert(idx, n)


def _quarter_tiles(qr):
    """Score tiles for quarter qr. Each tile is a list of pieces
    (j, psum_off, width, l_rel, diag): strip j's columns [l0+l_rel,
    l0+l_rel+width) land at tile columns [psum_off, psum_off+width).
    Full strips (below the diagonal) go 3 per tile; the 4 partial diagonal
    strips pack into one 1280-col tile, ordered 512|384|128|256 so every
    matmul output stays inside a single PSUM bank. The diag tile is emitted
    SECOND (when possible): its exp -> select -> AV chain is the longest
    serial path, and the following full tiles' QK/exp work hides it."""
    nfull = 4 * qr
    tiles = []
    for g0 in range(0, nfull, 3):
        grp = list(range(g0, min(g0 + 3, nfull)))
        tiles.append([(j, 512 * s, 512, 0, False) for s, j in enumerate(grp)])
    d = 4 * qr
    diag = [
        (d + 0, 0, 512, 0, True),
        (d + 1, 512, 384, 128, True),
        (d + 3, 896, 128, 384, True),
        (d + 2, 1024, 256, 256, True),
    ]
    tiles.insert(min(1, len(tiles)), diag)
    return tiles


def _build_program():
    nc = bass.Bass("TRN2", target_bir_lowering=False, debug=False)
    x_d = nc.declare_dram_parameter("x", [BH_PER_CORE, 65, QK_COLS], BF16, isOutput=False)
    v_d = nc.declare_dram_parameter("v", [BH_PER_CORE, 128, VP_COLS], BF16, isOutput=False)
    # output stays in the transposed orientation: [bh, quarter, d, l_rel];
    # row d == D is the softmax denominator; the host divides + transposes.
    o_d = nc.declare_dram_parameter("o", [BH_PER_CORE, 4, D + 1, 512], BF16, isOutput=True)

    with _SplitDrainTileContext(nc) as tc:
        with (
            tc.tile_pool(name="xin", bufs=1) as in_pool,
            tc.tile_pool(name="p", bufs=4) as p_pool,
            tc.tile_pool(name="otsb", bufs=2) as otsb_pool,
            tc.tile_pool(name="warm", bufs=1) as warm_pool,
            tc.tile_pool(name="strip_ps", bufs=2, space="PSUM") as strip_ps_pool,
            tc.tile_pool(name="out_ps", bufs=2, space="PSUM") as out_ps_pool,
        ):
            # dummy exp so the ~2.7us ACT_TABLE_LOAD overlaps the input DMA
            wsrc = warm_pool.tile([1, 8], F32, name="wsrc", tag="wsrc")
            wdst = warm_pool.tile([1, 8], F32, name="wdst", tag="wdst")
            nc.gpsimd.memset(wsrc, 0.0)
            nc.scalar.activation(wdst, wsrc, mybir.ActivationFunctionType.Exp)
            # lower-triangle keep-mask (1 where l >= s) for the DVE half of
            # the diagonal-block masking
            tri = warm_pool.tile([128, 128], BF16, name="tri", tag="tri")
            nc.gpsimd.memset(tri, 1.0)
            nc.gpsimd.affine_select(
                out=tri, in_=tri, compare_op=mybir.AluOpType.is_ge, fill=0.0,
                base=0, channel_multiplier=-1, pattern=[[1, 128]],
            )

            # prefetch all per-bh inputs; bh0's chunks are queued first so
            # compute starts as soon as they land (HWDGE drains FIFO)
            xqs, vps = [], []
            for i in range(BH_PER_CORE):
                xq = in_pool.tile([65, QK_COLS], BF16, name=f"xq{i}", tag=f"xq{i}")
                vp = in_pool.tile([128, VP_COLS], BF16, name=f"vp{i}", tag=f"vp{i}")
                xqs.append(xq)
                vps.append(vp)
            # bh0 is split into first-needed chunks (qt/kt cols 0:512, first
            # vp strips) so its first tiles unblock after ~130KB of traffic
            nc.sync.dma_start(out=xqs[0][:, 0:512], in_=x_d[0, :, 0:512])
            nc.sync.dma_start(out=xqs[0][:, L:L + 512], in_=x_d[0, :, L:L + 512])
            nc.sync.dma_start(out=vps[0][:, 0:4 * 65], in_=v_d[0, :, 0:4 * 65])
            nc.sync.dma_start(out=xqs[0][:, 512:L], in_=x_d[0, :, 512:L])
            nc.sync.dma_start(out=xqs[0][:, L + 512:QK_COLS], in_=x_d[0, :, L + 512:QK_COLS])
            nc.sync.dma_start(out=vps[0][:, 4 * 65:VP_COLS], in_=v_d[0, :, 4 * 65:VP_COLS])
            for i in range(1, BH_PER_CORE):
                nc.sync.dma_start(out=xqs[i][:, 0:L], in_=x_d[i, :, 0:L])
                nc.sync.dma_start(out=xqs[i][:, L:QK_COLS], in_=x_d[i, :, L:QK_COLS])
                nc.sync.dma_start(out=vps[i], in_=v_d[i])

            # Flat list of score-tile steps across all bh/quarters. Emission
            # is software-pipelined: at step T we emit QK(T), EXP(T-1),
            # selects(T-1), AV(T-2). QK running one tile ahead of EXP matches
            # the 2 strip PSUM bufs exactly, and AV lagging one tile means the
            # PE never stalls the ACT stream waiting on diag selects.
            steps = []
            for i in range(BH_PER_CORE):
                for qr in range(4):
                    tiles = _quarter_tiles(qr)
                    for ti, pieces in enumerate(tiles):
                        steps.append({
                            "bh": i, "qr": qr, "pieces": pieces,
                            "first": ti == 0, "last": ti == len(tiles) - 1,
                        })

            def emit_qk(st):
                i, qr, pieces = st["bh"], st["qr"], st["pieces"]
                qt = xqs[i][:, 0:L]
                kt = xqs[i][:, L:QK_COLS]
                l0 = 512 * qr
                xt_ps = strip_ps_pool.tile([128, TILE_W], F32)
                for (j, off, w, l_rel, diag) in pieces:
                    nc.tensor.matmul(
                        xt_ps[:, off:off + w],
                        lhsT=kt[:, 128 * j:128 * j + 128],
                        rhs=qt[:, l0 + l_rel:l0 + l_rel + w],
                        start=True, stop=True,
                    )
                st["xt_ps"] = xt_ps

            def emit_exp(st):
                pieces = st["pieces"]
                tw = pieces[-1][1] + pieces[-1][2]
                p = p_pool.tile([128, TILE_W], BF16)
                nc.scalar.activation(
                    p[:, 0:tw], st["xt_ps"][:, 0:tw],
                    mybir.ActivationFunctionType.Exp,
                )
                nsel = 0
                for (j, off, w, l_rel, diag) in pieces:
                    if diag:
                        # diagonal block: zero p where s > l (keep where
                        # (l - s) >= 0); alternate engines so the four
                        # selects run two-deep
                        if nsel % 2 == 0:
                            nc.gpsimd.affine_select(
                                out=p[:, off:off + 128],
                                in_=p[:, off:off + 128],
                                compare_op=mybir.AluOpType.is_ge, fill=0.0,
                                base=0, channel_multiplier=-1,
                                pattern=[[1, 128]],
                            )
                        else:
                            nc.vector.tensor_mul(
                                p[:, off:off + 128], p[:, off:off + 128], tri,
                            )
                        nsel += 1
                st["p"] = p

            ot_ps = [None]

            def emit_av(st):
                i, qr, pieces, p = st["bh"], st["qr"], st["pieces"], st["p"]
                if st["first"]:
                    ot_ps[0] = out_ps_pool.tile([D + 1, 512], F32, name="ot_ps")
                for n, (j, off, w, l_rel, diag) in enumerate(pieces):
                    nc.tensor.matmul(
                        ot_ps[0][:, l_rel:l_rel + w],
                        lhsT=vps[i][:, 65 * j:65 * j + 65],
                        rhs=p[:, off:off + w],
                        start=(st["first"] and n == 0),
                        stop=(st["last"] and n == len(pieces) - 1),
                    )
                if st["last"]:
                    # epilogue: evacuate PSUM as bf16 and ship raw (numerator
                    # rows + denominator row); the host divides + transposes.
                    ot_sb = otsb_pool.tile([D + 1, 512], BF16)
                    nc.vector.tensor_copy(ot_sb, ot_ps[0])
                    nc.sync.dma_start(out=o_d[i, qr], in_=ot_sb)

            for t, st in enumerate(steps):
                emit_qk(st)
                if t >= 1:
                    emit_exp(steps[t - 1])
                if t >= 2:
                    emit_av(steps[t - 2])
            emit_exp(steps[-1])
            emit_av(steps[-2])
            emit_av(steps[-1])
    _legalize_waits(nc)
    return nc


_PROGRAM = None


def _get_program():
    global _PROGRAM
    if _PROGRAM is None:
        _PROGRAM = _build_program()
    return _PROGRAM


def _prepare_inputs(q, k, v, tau, delta):
    """Pack full inputs into the per-core device layout (bf16)."""
    qs = (q.astype(np.float64) * (SCALE * tau.astype(np.float64))[:, 0, None, None, None]).astype(np.float32)
    # [B,L,H,E] -> [BH, E, L]
    qt = np.ascontiguousarray(qs.transpose(0, 2, 3, 1).reshape(BH, E, L))
    kt = np.ascontiguousarray(k.transpose(0, 2, 3, 1).reshape(BH, E, S))
    # V' = [v, 1]: [BH, S, D+1] -> [BH, 128, 16*(D+1)]
    vt = v.transpose(0, 2, 1, 3).reshape(BH, S, D)
    vp = np.concatenate([vt, np.ones((BH, S, 1), np.float32)], axis=2)
    vp = np.ascontiguousarray(
        vp.reshape(BH, S // 128, 128, D + 1).transpose(0, 2, 1, 3).reshape(BH, 128, VP_COLS)
    )
    dsc = (SCALE * delta).astype(np.float32)  # [B, S]

    x = np.empty((BH, E + 1, QK_COLS), np.float32)
    x[:, 0:E, 0:L] = qt
    x[:, E, 0:L] = 1.0
    x[:, 0:E, L:2 * L] = kt
    x[:, E, L:2 * L] = np.repeat(dsc, H, axis=0)
    return x.astype(NP_BF16), vp.astype(NP_BF16)


def _numpy_fallback(q, k, v, att_mask, tau, delta):
    out = np.empty((B, L, H, D), np.float32)
    mask = att_mask[:, 0]  # [B, L, S]
    for b in range(B):
        for h in range(H):
            s = (q[b, :, h, :] @ k[b, :, h, :].T) * tau[b, 0] + delta[b][None, :]
            s = np.where(mask[b], -1e9, s).astype(np.float32)
            s = SCALE * s
            s = s - s.max(axis=-1, keepdims=True)
            e = np.exp(s)
            a = e / e.sum(axis=-1, keepdims=True)
            out[b, :, h, :] = a @ v[b, :, h, :]
    return out


def kernel(q, k, v, att_mask, tau, delta):
    q = np.asarray(q, np.float32)
    k = np.asarray(k, np.float32)
    v = np.asarray(v, np.float32)
    tau = np.asarray(tau, np.float32)
    delta = np.asarray(delta, np.float32)
    att_mask = np.asarray(att_mask)

    causal = np.triu(np.ones((L, S), bool), k=1)
    if not all(np.array_equal(att_mask[b, 0], causal) for b in range(B)):
        return _numpy_fallback(q, k, v, att_mask, tau, delta)

    x, vp = _prepare_inputs(q, k, v, tau, delta)
    nc = _get_program()
    in_maps = [
        {
            "x": np.ascontiguousarray(x[c * BH_PER_CORE:(c + 1) * BH_PER_CORE]),
            "v": np.ascontiguousarray(vp[c * BH_PER_CORE:(c + 1) * BH_PER_CORE]),
        }
        for c in range(NCORES)
    ]
    res = run_bass_kernel_spmd(nc, in_maps, list(range(NCORES))).results

    out = np.empty((B, L, H, D), np.float32)
    for c in range(NCORES):
        o = np.asarray(res[c]["o"], dtype=np.float32)  # [4, 4, D+1, 512]
        norm = o[:, :, 0:D, :] / o[:, :, D:D + 1, :]
        for i in range(BH_PER_CORE):
            bh = c * BH_PER_CORE + i
            out[bh // H, :, bh % H, :] = norm[i].transpose(0, 2, 1).reshape(L, D)
    return out


# revision 11
# speedup vs baseline: 1.1748x; 1.1748x over previous
"""DSAttention (de-stationary attention) TRN2 Bass kernel.

Computes, per (b, h):
    scores = (q @ k^T) * tau_b + delta_b          [L, S]
    scores = where(causal_mask, -1e9, scores)
    A = softmax(scale * scores)                    (no max-subtraction needed:
                                                    logits are O(10), exp is safe)
    out = A @ v                                    [L, D]

Strategy: batch*head parallel over 8 cores (4 (b,h) pairs per core).
Per (b,h), everything is computed in "transposed score" space:
    X_T[s, l] = sum_e KT[e, s] * QT[e, l]       (QT pre-scaled by scale*tau on host)
    p = exp(X_T)                                (delta folded in via an aug row:
                                                 KT row 64 = scale*delta, QT row 64 = 1)
    OutT[d, l] = sum_s V'[s, d] * p[s, l]       (V' has a ones column -> row 64 of
                                                 OutT is the softmax denominator)
then the host divides + transposes.

The kernel is ACT(exp)-throughput-bound: the Scalar engine evaluates one exp
per lane per cycle at 1.2 GHz, dtype-independent, and every unmasked score
needs one exp. Everything else is arranged to keep ACT saturated:
  - bf16 data path (matmul runs at full PE rate; halves DMA + SBUF traffic)
  - contraction K=65 (64 e-dims + aug row) -> no zero-pad rows, no memsets,
    input DMA can start immediately and compute starts ~4us in
  - causality exploited at 128-column granularity with NO junk columns:
    the four partial diagonal strips are packed contiguously into one PSUM
    tile (order 512|384|128|256 keeps each matmul inside one 2KB bank)
  - exp is issued over 3-bank PSUM tiles (up to 1536 cols per ACTIVATE,
    52 calls/core instead of 80) to amortize the ~150ns per-call overhead
  - a dummy exp at t=0 pulls the ~2.7us ACT_TABLE_LOAD into the DMA wait
"""

import math

import numpy as np
import ml_dtypes

import bass_rust
import concourse.bass as bass
import concourse.mybir as mybir
import concourse.tile as tile
from concourse.bass_utils import run_bass_kernel_spmd

B, L, S, H, E, D = 2, 2048, 2048, 16, 64, 64
NCORES = 8
BH = B * H                      # 32 (b,h) pairs
BH_PER_CORE = BH // NCORES      # 4
SCALE = 1.0 / math.sqrt(E)

F32 = mybir.dt.float32
F32R = mybir.dt.float32r
BF16 = mybir.dt.bfloat16
NP_BF16 = ml_dtypes.bfloat16

# per-(b,h) input layout:
#   x  [65, 4096]  cols [0, 2048)    rows 0:64 = scale*tau*q^T, row 64 = 1.0
#                  cols [2048, 4096) rows 0:64 = k^T, row 64 = scale*delta
#   v  [128, 1040] V' rearranged: col 65*j + c = V'[128j+p, c]
QK_COLS = 2 * L                 # 4096
VP_COLS = (S // 128) * (D + 1)  # 1040
TILE_W = 1536                   # 3 PSUM banks per score tile


class _SplitDrainTileContext(tile.TileContext):
    """This walrus build rejects instructions carrying more than one sem
    wait; the kernel-tail drain aggregates one wait per active processor.
    Split them across a chain of drains on the same engine."""

    def _drain_and_barrier(self, tick_clock, wait_clock):
        nc = self.nc
        drain_inst = nc.sync.drain()
        wait_clock.add_sem_waits(
            drain_inst.ins, bass_rust.ScopedClock({None: tick_clock.global_clock})
        )
        si = drain_inst.ins.sync_info
        waits = list(si.on_wait) if si is not None and si.on_wait else []
        if len(waits) > 1:
            si.on_wait = waits[:1]
            for w in waits[1:]:
                d2 = nc.sync.drain()
                d2.ins.sync_info = bass_rust.SyncInfo(on_wait=[w], on_update=[])
        nc.all_engine_barrier()
        popped = nc._tile_sem_poison_stack.pop()
        assert popped is self._sem_poison
        nc.clear_and_free_semaphores(list(self.sems.allocated().values()))
        nc.all_engine_barrier()


def _legalize_waits(nc, max_waits=1):
    """This walrus build rejects instructions with more than `max_waits`
    sem waits. Spill extras onto same-engine NoOps inserted just before
    the offending instruction (same-engine program order preserves the
    wait semantics)."""
    for f in nc.m.functions:
        for bb in f.blocks:
            insts = bb.instructions
            for idx in range(len(insts) - 1, -1, -1):
                inst = insts[idx]
                si = getattr(inst, "sync_info", None)
                if si is None or not si.on_wait:
                    continue
                ow = list(si.on_wait)
                sem = [w for w in ow if w.sync_type == "semaphore"]
                other = [w for w in ow if w.sync_type != "semaphore"]
                budget = max(0, max_waits - len(other))
                if len(sem) <= budget:
                    continue
                keep, spill = sem[:budget], sem[budget:]
                si.on_wait = other + keep
                for w in reversed(spill):
                    n = mybir.InstNoOp(name=f"W-{nc.next_id()}", ins=[], outs=[])
                    n.engine = inst.engine
                    n.sync_info = bass_rust.SyncInfo(on_wait=[w], on_update=[])
                    nc.register_instruction(n, overwrite=True)
                    insts.insert(idx, n)


def _quarter_tiles(qr):
    """Score tiles for quarter qr. Each tile is a list of pieces
    (j, psum_off, width, l_rel, diag): strip j's columns [l0+l_rel,
    l0+l_rel+width) land at tile columns [psum_off, psum_off+width).
    Full strips (below the diagonal) go 3 per tile; the 4 partial diagonal
    strips pack into one 1280-col tile, ordered 512|384|128|256 so every
    matmul output stays inside a single PSUM bank. The diag tile is emitted
    SECOND (when possible): its exp -> select -> AV chain is the longest
    serial path, and the following full tiles' QK/exp work hides it."""
    nfull = 4 * qr
    tiles = []
    for g0 in range(0, nfull, 3):
        grp = list(range(g0, min(g0 + 3, nfull)))
        tiles.append([(j, 512 * s, 512, 0, False) for s, j in enumerate(grp)])
    d = 4 * qr
    diag = [
        (d + 0, 0, 512, 0, True),
        (d + 1, 512, 384, 128, True),
        (d + 3, 896, 128, 384, True),
        (d + 2, 1024, 256, 256, True),
    ]
    tiles.insert(min(1, len(tiles)), diag)
    return tiles


def _build_program():
    nc = bass.Bass("TRN2", target_bir_lowering=False, debug=False)
    x_d = nc.declare_dram_parameter("x", [BH_PER_CORE, 65, QK_COLS], BF16, isOutput=False)
    v_d = nc.declare_dram_parameter("v", [BH_PER_CORE, 128, VP_COLS], F32R, isOutput=False)
    # output stays in the transposed orientation: [bh, quarter, d, l_rel];
    # row d == D is the softmax denominator; the host divides + transposes.
    o_d = nc.declare_dram_parameter("o", [BH_PER_CORE, 4, D + 1, 512], BF16, isOutput=True)

    with _SplitDrainTileContext(nc) as tc:
        with (
            tc.tile_pool(name="xin", bufs=1) as in_pool,
            tc.tile_pool(name="p", bufs=4) as p_pool,
            tc.tile_pool(name="otsb", bufs=2) as otsb_pool,
            tc.tile_pool(name="warm", bufs=1) as warm_pool,
            tc.tile_pool(name="strip_ps", bufs=2, space="PSUM") as strip_ps_pool,
            tc.tile_pool(name="out_ps", bufs=2, space="PSUM") as out_ps_pool,
        ):
            # dummy exp so the ~2.7us ACT_TABLE_LOAD overlaps the input DMA
            wsrc = warm_pool.tile([1, 8], F32, name="wsrc", tag="wsrc")
            wdst = warm_pool.tile([1, 8], F32, name="wdst", tag="wdst")
            nc.gpsimd.memset(wsrc, 0.0)
            nc.scalar.activation(wdst, wsrc, mybir.ActivationFunctionType.Exp)
            # lower-triangle keep-mask (1 where l >= s) for the DVE half of
            # the diagonal-block masking
            tri = warm_pool.tile([128, 128], F32R, name="tri", tag="tri")
            # uint32 view dodges the f32r ISA check (0x3F800000 == 1.0f)
            nc.gpsimd._memset_packed(tri.bitcast(mybir.dt.uint32), 0x3F800000)
            nc.gpsimd.affine_select(
                out=tri, in_=tri, compare_op=mybir.AluOpType.is_ge, fill=0.0,
                base=0, channel_multiplier=-1, pattern=[[1, 128]],
            )

            # prefetch all per-bh inputs; bh0's chunks are queued first so
            # compute starts as soon as they land (HWDGE drains FIFO)
            xqs, vps = [], []
            for i in range(BH_PER_CORE):
                xq = in_pool.tile([65, QK_COLS], BF16, name=f"xq{i}", tag=f"xq{i}")
                vp = in_pool.tile([128, VP_COLS], F32R, name=f"vp{i}", tag=f"vp{i}")
                xqs.append(xq)
                vps.append(vp)
            # bh0 is split into first-needed chunks (qt/kt cols 0:512, first
            # vp strips) so its first tiles unblock after ~130KB of traffic
            nc.sync.dma_start(out=xqs[0][:, 0:512], in_=x_d[0, :, 0:512])
            nc.sync.dma_start(out=xqs[0][:, L:L + 512], in_=x_d[0, :, L:L + 512])
            nc.sync.dma_start(out=vps[0][:, 0:4 * 65], in_=v_d[0, :, 0:4 * 65])
            nc.sync.dma_start(out=xqs[0][:, 512:L], in_=x_d[0, :, 512:L])
            nc.sync.dma_start(out=xqs[0][:, L + 512:QK_COLS], in_=x_d[0, :, L + 512:QK_COLS])
            nc.sync.dma_start(out=vps[0][:, 4 * 65:VP_COLS], in_=v_d[0, :, 4 * 65:VP_COLS])
            for i in range(1, BH_PER_CORE):
                nc.sync.dma_start(out=xqs[i][:, 0:L], in_=x_d[i, :, 0:L])
                nc.sync.dma_start(out=xqs[i][:, L:QK_COLS], in_=x_d[i, :, L:QK_COLS])
                nc.sync.dma_start(out=vps[i], in_=v_d[i])

            # Flat list of score-tile steps across all bh/quarters. Emission
            # is software-pipelined: at step T we emit QK(T), EXP(T-1),
            # selects(T-1), AV(T-2). QK running one tile ahead of EXP matches
            # the 2 strip PSUM bufs exactly, and AV lagging one tile means the
            # PE never stalls the ACT stream waiting on diag selects.
            steps = []
            for i in range(BH_PER_CORE):
                for qr in range(4):
                    tiles = _quarter_tiles(qr)
                    for ti, pieces in enumerate(tiles):
                        steps.append({
                            "bh": i, "qr": qr, "pieces": pieces,
                            "first": ti == 0, "last": ti == len(tiles) - 1,
                        })

            def emit_qk(st):
                i, qr, pieces = st["bh"], st["qr"], st["pieces"]
                qt = xqs[i][:, 0:L]
                kt = xqs[i][:, L:QK_COLS]
                l0 = 512 * qr
                xt_ps = strip_ps_pool.tile([128, TILE_W], F32)
                for (j, off, w, l_rel, diag) in pieces:
                    nc.tensor.matmul(
                        xt_ps[:, off:off + w],
                        lhsT=kt[:, 128 * j:128 * j + 128],
                        rhs=qt[:, l0 + l_rel:l0 + l_rel + w],
                        start=True, stop=True,
                    )
                st["xt_ps"] = xt_ps

            def emit_exp(st):
                pieces = st["pieces"]
                tw = pieces[-1][1] + pieces[-1][2]
                p = p_pool.tile([128, TILE_W], F32R)
                nc.scalar.activation(
                    p[:, 0:tw], st["xt_ps"][:, 0:tw],
                    mybir.ActivationFunctionType.Exp,
                )
                nsel = 0
                for (j, off, w, l_rel, diag) in pieces:
                    if diag:
                        # diagonal block: zero p where s > l (keep where
                        # (l - s) >= 0); alternate engines so the four
                        # selects run two-deep
                        if nsel % 2 == 0:
                            nc.gpsimd.affine_select(
                                out=p[:, off:off + 128],
                                in_=p[:, off:off + 128],
                                compare_op=mybir.AluOpType.is_ge, fill=0.0,
                                base=0, channel_multiplier=-1,
                                pattern=[[1, 128]],
                            )
                        else:
                            nc.vector.tensor_mul(
                                p[:, off:off + 128], p[:, off:off + 128], tri,
                            )
                        nsel += 1
                st["p"] = p

            ot_ps = [None]

            def emit_av(st):
                i, qr, pieces, p = st["bh"], st["qr"], st["pieces"], st["p"]
                if st["first"]:
                    ot_ps[0] = out_ps_pool.tile([D + 1, 512], F32, name="ot_ps")
                for n, (j, off, w, l_rel, diag) in enumerate(pieces):
                    nc.tensor.matmul(
                        ot_ps[0][:, l_rel:l_rel + w],
                        lhsT=vps[i][:, 65 * j:65 * j + 65],
                        rhs=p[:, off:off + w],
                        start=(st["first"] and n == 0),
                        stop=(st["last"] and n == len(pieces) - 1),
                    )
                if st["last"]:
                    # epilogue: evacuate PSUM as bf16 and ship raw (numerator
                    # rows + denominator row); the host divides + transposes.
                    ot_sb = otsb_pool.tile([D + 1, 512], BF16)
                    nc.vector.tensor_copy(ot_sb, ot_ps[0])
                    nc.sync.dma_start(out=o_d[i, qr], in_=ot_sb)

            for t, st in enumerate(steps):
                emit_qk(st)
                if t >= 1:
                    emit_exp(steps[t - 1])
                if t >= 2:
                    emit_av(steps[t - 2])
            emit_exp(steps[-1])
            emit_av(steps[-2])
            emit_av(steps[-1])
    _legalize_waits(nc)
    return nc


_PROGRAM = None


def _get_program():
    global _PROGRAM
    if _PROGRAM is None:
        _PROGRAM = _build_program()
    return _PROGRAM


def _round_f32r(a):
    """Round fp32 to the f32r grid (13 low mantissa bits zeroed, RNE) --
    matches the PE's fp32r operand format."""
    b = a.astype(np.float32).view(np.uint32)
    r = (b + np.uint32(0x0FFF) + ((b >> np.uint32(13)) & np.uint32(1))) & ~np.uint32(0x1FFF)
    return r.view(np.float32)


def _prepare_inputs(q, k, v, tau, delta):
    """Pack full inputs into the per-core device layout (bf16)."""
    qs = (q.astype(np.float64) * (SCALE * tau.astype(np.float64))[:, 0, None, None, None]).astype(np.float32)
    # [B,L,H,E] -> [BH, E, L]
    qt = np.ascontiguousarray(qs.transpose(0, 2, 3, 1).reshape(BH, E, L))
    kt = np.ascontiguousarray(k.transpose(0, 2, 3, 1).reshape(BH, E, S))
    # V' = [v, 1]: [BH, S, D+1] -> [BH, 128, 16*(D+1)]
    vt = v.transpose(0, 2, 1, 3).reshape(BH, S, D)
    vp = np.concatenate([vt, np.ones((BH, S, 1), np.float32)], axis=2)
    vp = np.ascontiguousarray(
        vp.reshape(BH, S // 128, 128, D + 1).transpose(0, 2, 1, 3).reshape(BH, 128, VP_COLS)
    )
    dsc = (SCALE * delta).astype(np.float32)  # [B, S]

    x = np.empty((BH, E + 1, QK_COLS), np.float32)
    x[:, 0:E, 0:L] = qt
    x[:, E, 0:L] = 1.0
    x[:, 0:E, L:2 * L] = kt
    x[:, E, L:2 * L] = np.repeat(dsc, H, axis=0)
    return x.astype(NP_BF16), _round_f32r(vp)


def _numpy_fallback(q, k, v, att_mask, tau, delta):
    out = np.empty((B, L, H, D), np.float32)
    mask = att_mask[:, 0]  # [B, L, S]
    for b in range(B):
        for h in range(H):
            s = (q[b, :, h, :] @ k[b, :, h, :].T) * tau[b, 0] + delta[b][None, :]
            s = np.where(mask[b], -1e9, s).astype(np.float32)
            s = SCALE * s
            s = s - s.max(axis=-1, keepdims=True)
            e = np.exp(s)
            a = e / e.sum(axis=-1, keepdims=True)
            out[b, :, h, :] = a @ v[b, :, h, :]
    return out


def kernel(q, k, v, att_mask, tau, delta):
    q = np.asarray(q, np.float32)
    k = np.asarray(k, np.float32)
    v = np.asarray(v, np.float32)
    tau = np.asarray(tau, np.float32)
    delta = np.asarray(delta, np.float32)
    att_mask = np.asarray(att_mask)

    causal = np.triu(np.ones((L, S), bool), k=1)
    if not all(np.array_equal(att_mask[b, 0], causal) for b in range(B)):
        return _numpy_fallback(q, k, v, att_mask, tau, delta)

    x, vp = _prepare_inputs(q, k, v, tau, delta)
    nc = _get_program()
    in_maps = [
        {
            "x": np.ascontiguousarray(x[c * BH_PER_CORE:(c + 1) * BH_PER_CORE]),
            "v": np.ascontiguousarray(vp[c * BH_PER_CORE:(c + 1) * BH_PER_CORE]),
        }
        for c in range(NCORES)
    ]
    res = run_bass_kernel_spmd(nc, in_maps, list(range(NCORES))).results

    out = np.empty((B, L, H, D), np.float32)
    for c in range(NCORES):
        o = np.asarray(res[c]["o"], dtype=np.float32)  # [4, 4, D+1, 512]
        norm = o[:, :, 0:D, :] / o[:, :, D:D + 1, :]
        for i in range(BH_PER_CORE):
            bh = c * BH_PER_CORE + i
            out[bh // H, :, bh % H, :] = norm[i].transpose(0, 2, 1).reshape(L, D)
    return out
